# revision 18
# baseline (speedup 1.0000x reference)
"""Trainium2 Bass kernel for nn_MultiHeadAttention_59863254172512.

Sharding: 8 cores = 2 batches x 4 query-row blocks of 512. Each core computes
the full attention for its (batch, 512 query rows): k/v projections for the
whole batch are recomputed per core (no cross-core communication needed).

Layout strategy per core:
  - activations are DMA-transposed (XBAR, bf16) into [D-part, L] tiles
  - q/k projected into transposed layout [HDK-part, L]; v_s/v_o into natural
    [LK-part, DV] (v_s augmented with a ones column per head for the softmax
    denominator)
  - scores computed transposed [LK-part, LQ]; exp on ScalarE with the
    per-key bias and 1/16 scale folded in; mask applied multiplicatively
  - attn@v contracts keys on partitions; denominator rides the ones column
  - normalization deferred (linearity of fc) to the o^T tiles via a
    DRAM-bounce partition-broadcast of 1/den
  - fc output lands natural [LQ-part, D]; residual add + LayerNorm on-chip
"""

from contextlib import ExitStack

import numpy as np
import ml_dtypes

import concourse.bass as bass
import concourse.mybir as mybir
import concourse.tile as tile
from concourse import bacc, bass_utils

B, LQ, LK, D = 2, 2048, 2048, 1024
H, DK, DV = 16, 64, 64
EPS = 1e-6
P = 128
N_CORES = 8
LQL = LQ * B // N_CORES          # 512 query rows per core
DC = D // P                      # 8 contraction chunks
KC = LK // P                     # 16 key chunks
HC = (H * DK) // P               # 8 hdk chunks (2 heads per chunk)
LCH = LQL // P                   # 4 local query row chunks
F32 = mybir.dt.float32
BF16 = mybir.dt.bfloat16

_CACHE = {}
DEBUG = False


def _build_nc():
    nc = bacc.Bacc("TRN2", target_bir_lowering=False, debug=False,
                   num_devices=N_CORES)

    # ---- I/O ----
    gs_bf = nc.dram_tensor("gs_bf", [LQL, D], BF16, kind="ExternalInput")
    xk_bf = nc.dram_tensor("xk_bf", [LK, D], BF16, kind="ExternalInput")
    xs_bf = nc.dram_tensor("xs_bf", [LK, D], BF16, kind="ExternalInput")
    xo_bf = nc.dram_tensor("xo_bf", [LK, D], BF16, kind="ExternalInput")
    mask_bf = nc.dram_tensor("mask_bf", [LQL, LK], BF16, kind="ExternalInput")
    hbias = nc.dram_tensor("hbias", [LK], F32, kind="ExternalInput")
    resid = nc.dram_tensor("resid", [LQL, D], F32, kind="ExternalInput")
    wq_bf = nc.dram_tensor("wq_bf", [D, H * DK], BF16, kind="ExternalInput")
    wk_bf = nc.dram_tensor("wk_bf", [D, H * DK], BF16, kind="ExternalInput")
    wv_bf = nc.dram_tensor("wv_bf", [D, H * DV], BF16, kind="ExternalInput")
    wfc_bf = nc.dram_tensor("wfc_bf", [H * DV, D], BF16, kind="ExternalInput")
    gamma = nc.dram_tensor("gamma", [D], F32, kind="ExternalInput")
    beta = nc.dram_tensor("beta", [D], F32, kind="ExternalInput")
    out_s = nc.dram_tensor("out_s", [LQL, D], F32, kind="ExternalOutput")
    out_o = nc.dram_tensor("out_o", [LQL, D], F32, kind="ExternalOutput")
    if DEBUG:
        dbg_den = nc.dram_tensor("dbg_den", [H, LQL], F32, kind="ExternalOutput")
        dbg_os = nc.dram_tensor("dbg_os", [P, HC, LQL], BF16, kind="ExternalOutput")
        dbg_q = nc.dram_tensor("dbg_q", [P, HC, LQL], BF16, kind="ExternalOutput")

    with tile.TileContext(nc) as tc, ExitStack() as ctx:
        if True:
            const_p = ctx.enter_context(tc.tile_pool(name="const", bufs=1))
            xt_p = ctx.enter_context(tc.tile_pool(name="xt", bufs=3))
            w_p = ctx.enter_context(tc.tile_pool(name="w", bufs=2))
            pers_p = ctx.enter_context(tc.tile_pool(name="persist", bufs=1))
            aug_p = ctx.enter_context(tc.tile_pool(name="aug", bufs=2))
            vh_p = ctx.enter_context(tc.tile_pool(name="vh", bufs=2))
            attn_p = ctx.enter_context(tc.tile_pool(name="attn", bufs=6))
            den_p = ctx.enter_context(tc.tile_pool(name="den", bufs=2))
            res_p = ctx.enter_context(tc.tile_pool(name="resp", bufs=1))
            ln_p = ctx.enter_context(tc.tile_pool(name="lnp", bufs=3))
            stats_p = ctx.enter_context(tc.tile_pool(name="stats", bufs=4))
            ps_mm = ctx.enter_context(tc.tile_pool(name="psmm", bufs=2, space="PSUM"))
            ps_sc = ctx.enter_context(tc.tile_pool(name="pssc", bufs=2, space="PSUM"))
            ps_acc = ctx.enter_context(tc.tile_pool(name="psacc", bufs=2, space="PSUM"))
            dram_p = ctx.enter_context(tc.tile_pool(name="dram", bufs=1, space="DRAM"))
            dram_den = ctx.enter_context(tc.tile_pool(name="dramd", bufs=2, space="DRAM"))
            # ---- persistent attention operands ----
            kT = pers_p.tile([P, HC, LK], BF16)      # 4 MiB
            qT = pers_p.tile([P, HC, LQL], BF16)     # 1 MiB
            maskT = pers_p.tile([P, KC, LQL], BF16)  # 2 MiB
            osT = pers_p.tile([P, HC, LQL], BF16)    # 1 MiB
            ooT = pers_p.tile([P, HC, LQL], BF16)    # 1 MiB

            # DRAM scratch for v (streamed back per head)
            vsD = dram_p.tile([KC, P, H * DV], BF16)
            voD = dram_p.tile([KC, P, H * DV], BF16)


            class XTPair:
                def __init__(self, a, b):
                    self._t = (a, b)

                def __getitem__(self, key):
                    p, dc, fs = key
                    return self._t[dc // 4][p, dc % 4, fs]

            def load_xt(src):
                t0 = xt_p.tile([P, 4, LK], BF16, tag="xt", name="xt_t")
                for dc in range(4):
                    nc.sync.dma_start_transpose(
                        t0[:, dc, :], src[:, dc * P:(dc + 1) * P])
                t1 = xt_p.tile([P, 4, LK], BF16, tag="xt", name="xt_t")
                for dc in range(4, DC):
                    nc.sync.dma_start_transpose(
                        t1[:, dc - 4, :], src[:, dc * P:(dc + 1) * P])
                return XTPair(t0, t1)

            def load_w(src, n):
                t = w_p.tile([P, DC, 1024], BF16, tag="w", name="w_t")
                nc.gpsimd.dma_start(t[:], src.rearrange("(o p) n -> p o n", p=P))
                return t

            # ---- phase B: projections ----
            wk_sb = load_w(wk_bf, "wk")
            xkT = load_xt(xk_bf)
            wq_sb = load_w(wq_bf, "wq")
            xqT = xt_p.tile([P, DC, LQL], BF16, tag="xt", name="xqT")
            for dc in range(DC):
                nc.sync.dma_start_transpose(
                    xqT[:, dc, :], gs_bf[:, dc * P:(dc + 1) * P])
            for kc in range(KC):
                nc.sync.dma_start_transpose(
                    maskT[:, kc, :], mask_bf[:, kc * P:(kc + 1) * P])

            # k projection -> kT
            for hc in range(HC):
                for lc in range(LK // 512):
                    ps_prj = ps_mm.tile([P, 512], F32, tag="mm", name="ps_prj")
                    for dc in range(DC):
                        nc.tensor.matmul(
                            ps_prj[:],
                            wk_sb[:, dc, hc * P:(hc + 1) * P],
                            xkT[slice(None), dc, slice(lc * 512, (lc + 1) * 512)],
                            start=(dc == 0), stop=(dc == DC - 1))
                    nc.any.tensor_copy(out=kT[:, hc, lc * 512:(lc + 1) * 512],
                                       in_=ps_prj[:])

            # q projection -> qT (runs while XsT transposes stream in)
            for hc in range(HC):
                ps_prj = ps_mm.tile([P, 512], F32, tag="mm", name="ps_prj")
                for dc in range(DC):
                    nc.tensor.matmul(ps_prj[:, :LQL],
                                     wq_sb[:, dc, hc * P:(hc + 1) * P],
                                     xqT[:, dc, :],
                                     start=(dc == 0), stop=(dc == DC - 1))
                nc.any.tensor_copy(out=qT[:, hc, :], in_=ps_prj[:, :LQL])

            # v_s projection -> DRAM
            xsT = load_xt(xs_bf)
            wv_sb = load_w(wv_bf, "wv")
            for kc in range(KC):
                vs_t = aug_p.tile([P, H * DV], BF16, tag="vo_t", name="vs_t")
                for dvc in range(2):
                    ps_prj = ps_mm.tile([P, 512], F32, tag="mm", name="ps_prj")
                    for dc in range(DC):
                        nc.tensor.matmul(
                            ps_prj[:],
                            xsT[slice(None), dc, slice(kc * P, (kc + 1) * P)],
                            wv_sb[:, dc, dvc * 512:(dvc + 1) * 512],
                            start=(dc == 0), stop=(dc == DC - 1))
                    nc.any.tensor_copy(out=vs_t[:, dvc * 512:(dvc + 1) * 512],
                                       in_=ps_prj[:])
                nc.scalar.dma_start(vsD[kc], vs_t[:])

            # v_o projection -> DRAM
            xoT = load_xt(xo_bf)
            for kc in range(KC):
                vo_t = aug_p.tile([P, H * DV], BF16, tag="vo_t", name="vo_t")
                for dvc in range(2):
                    ps_prj = ps_mm.tile([P, 512], F32, tag="mm", name="ps_prj")
                    for dc in range(DC):
                        nc.tensor.matmul(
                            ps_prj[:],
                            xoT[slice(None), dc, slice(kc * P, (kc + 1) * P)],
                            wv_sb[:, dc, dvc * 512:(dvc + 1) * 512],
                            start=(dc == 0), stop=(dc == DC - 1))
                    nc.any.tensor_copy(out=vo_t[:, dvc * 512:(dvc + 1) * 512],
                                       in_=ps_prj[:])
                nc.scalar.dma_start(voD[kc], vo_t[:])

            wfc_sb = load_w(wfc_bf, "wfc")

            # ---- constants (needed from attention phase on) ----
            hb_sb = const_p.tile([P, KC], F32)
            nc.gpsimd.dma_start(hb_sb[:], hbias.rearrange("(o p) -> p o", p=P))
            gamma_bc = const_p.tile([P, D], F32)
            nc.gpsimd.dma_start(gamma_bc[:], gamma[:].partition_broadcast(P))
            beta_bc = const_p.tile([P, D], F32)
            nc.gpsimd.dma_start(beta_bc[:], beta[:].partition_broadcast(P))
            eps_sb = const_p.tile([P, 1], F32)
            nc.vector.memset(eps_sb[:], EPS)
            ones_sb = const_p.tile([P, 1], BF16)
            nc.vector.memset(ones_sb[:], 1.0)

            # ---- phase C: attention per head ----
            for h in range(H):
                hp, hr = h // 2, (h % 2) * 64
                sv_h = vh_p.tile([P, KC, 2 * DV], BF16)
                nc.sync.dma_start(
                    sv_h[:, :, 0:DV], vsD[:, :, h * DV:(h + 1) * DV]
                    .rearrange("k p c -> p k c"))
                nc.sync.dma_start(
                    sv_h[:, :, DV:2 * DV], voD[:, :, h * DV:(h + 1) * DV]
                    .rearrange("k p c -> p k c"))

                ps_sv = ps_acc.tile([P, 512], F32, tag="ps_sv", name="ps_sv")
                den_ps = ps_acc.tile([P, 4], F32, tag="den", name="den_ps")
                for kc in range(KC):
                    ps_scr = ps_sc.tile([P, 512], F32, tag="sc", name="ps_scr")
                    nc.tensor.matmul(
                        ps_scr[:, :LQL],
                        kT[hr:hr + 64, hp, kc * P:(kc + 1) * P],
                        qT[hr:hr + 64, hp, :],
                        start=True, stop=True)
                    attn_t = attn_p.tile([P, LQL], BF16)
                    nc.scalar.activation(
                        out=attn_t[:], in_=ps_scr[:, :LQL],
                        func=mybir.ActivationFunctionType.Exp,
                        bias=hb_sb[:, kc:kc + 1], scale=0.0625)
                    nc.vector.tensor_mul(
                        out=attn_t[:], in0=attn_t[:], in1=maskT[:, kc, :])
                    nc.tensor.matmul(ps_sv[:, :LQL], sv_h[:, kc, :], attn_t[:],
                                     start=(kc == 0), stop=(kc == KC - 1))
                    for qc in range(4):
                        nc.tensor.matmul(
                            den_ps[:, qc:qc + 1],
                            attn_t[:, qc * P:(qc + 1) * P], ones_sb[:],
                            start=(kc == 0 and qc == 0),
                            stop=(kc == KC - 1 and qc == 3))

                # denominator -> reciprocal -> broadcast over 64 partitions
                rc4 = den_p.tile([P, 4], F32)
                nc.vector.reciprocal(out=rc4[:], in_=den_ps[:])
                denD = dram_den.tile([LQL], F32)
                nc.sync.dma_start(
                    denD.rearrange("(q p) -> p q", p=P), rc4[:])
                recip_bc = den_p.tile([64, LQL], F32)
                nc.sync.dma_start(recip_bc[:], denD[:].partition_broadcast(64))

                nc.vector.tensor_mul(out=osT[hr:hr + 64, hp, :],
                                     in0=ps_sv[0:64, :LQL], in1=recip_bc[:])
                nc.vector.tensor_mul(out=ooT[hr:hr + 64, hp, :],
                                     in0=ps_sv[64:128, :LQL], in1=recip_bc[:])
                if DEBUG:
                    nc.sync.dma_start(dbg_den[h:h + 1, :],
                                      denD[:].partition_broadcast(1))

            if DEBUG:
                nc.sync.dma_start(dbg_q[:], qT[:])
                nc.sync.dma_start(dbg_os[:], osT[:])

            # ---- phase D: fc + residual + layernorm ----
            for lc in range(LCH):
                res_t = res_p.tile([P, D], F32)
                nc.gpsimd.dma_start(res_t[:], resid[lc * P:(lc + 1) * P, :])
                for variant, (oT, out_d) in enumerate(
                        [(osT, out_s), (ooT, out_o)]):
                    ln_t = ln_p.tile([P, D], F32)
                    for dc2 in range(2):
                        ps_fc = ps_mm.tile([P, 512], F32, tag="mm", name="ps_fc")
                        for hp2 in range(HC):
                            nc.tensor.matmul(
                                ps_fc[:],
                                oT[:, hp2, lc * P:(lc + 1) * P],
                                wfc_sb[:, hp2, dc2 * 512:(dc2 + 1) * 512],
                                start=(hp2 == 0), stop=(hp2 == HC - 1))
                        nc.vector.tensor_add(
                            out=ln_t[:, dc2 * 512:(dc2 + 1) * 512],
                            in0=ps_fc[:],
                            in1=res_t[:, dc2 * 512:(dc2 + 1) * 512])
                    # layernorm over D
                    stats = stats_p.tile([P, 2, nc.vector.BN_STATS_DIM], F32)
                    for sg in range(2):
                        nc.vector.bn_stats(
                            out=stats[:, sg, :],
                            in_=ln_t[:, sg * 512:(sg + 1) * 512])
                    mv = stats_p.tile([P, nc.vector.BN_AGGR_DIM], F32)
                    nc.vector.bn_aggr(out=mv[:], in_=stats[:])
                    rstd = stats_p.tile([P, 1], F32)
                    nc.scalar.activation(
                        out=rstd[:], in_=mv[:, 1:2],
                        func=mybir.ActivationFunctionType.Sqrt,
                        bias=eps_sb[:], scale=1.0)
                    nc.vector.reciprocal(out=rstd[:], in_=rstd[:])
                    nc.vector.tensor_scalar(
                        out=ln_t[:], in0=ln_t[:],
                        scalar1=mv[:, 0:1], scalar2=rstd[:],
                        op0=mybir.AluOpType.subtract,
                        op1=mybir.AluOpType.mult)
                    nc.vector.tensor_mul(out=ln_t[:], in0=ln_t[:],
                                         in1=gamma_bc[:])
                    nc.vector.tensor_add(out=ln_t[:], in0=ln_t[:],
                                         in1=beta_bc[:])
                    nc.sync.dma_start(out_d[lc * P:(lc + 1) * P, :], ln_t[:])

    nc.compile()
    return nc


def _in_maps(PE_states, global_state, PE_statements, PE_operators,
             PE_solution_scores, mask, w_q, w_k, w_v, w_fc, ln_gamma, ln_beta):
    bf = lambda a: np.ascontiguousarray(a).astype(ml_dtypes.bfloat16)
    f32 = lambda a: np.ascontiguousarray(a, dtype=np.float32)
    xk = [bf(PE_states[b]) for b in range(B)]
    xs = [bf(PE_statements[b]) for b in range(B)]
    xo = [bf(PE_operators[b]) for b in range(B)]
    hb = [f32(PE_solution_scores[b] * 0.5) for b in range(B)]
    wq, wk, wv, wfc = bf(w_q), bf(w_k), bf(w_v), bf(w_fc)
    g, bt = f32(ln_gamma), f32(ln_beta)
    maps = []
    for c in range(N_CORES):
        b, qs = c // 4, (c % 4) * LQL
        maps.append({
            "gs_bf": bf(global_state[b, qs:qs + LQL]),
            "xk_bf": xk[b], "xs_bf": xs[b], "xo_bf": xo[b],
            "mask_bf": bf(mask[b, qs:qs + LQL]),
            "hbias": hb[b],
            "resid": f32(global_state[b, qs:qs + LQL]),
            "wq_bf": wq, "wk_bf": wk, "wv_bf": wv, "wfc_bf": wfc,
            "gamma": g, "beta": bt,
        })
    return maps


def kernel(**inputs):
    if "nc" not in _CACHE:
        _CACHE["nc"] = _build_nc()
    nc = _CACHE["nc"]
    maps = _in_maps(**inputs)
    res = bass_utils.run_bass_kernel_spmd(nc, maps, core_ids=list(range(N_CORES)))
    sp = np.empty((B, LQ, D), np.float32)
    op = np.empty((B, LQ, D), np.float32)
    for c in range(N_CORES):
        b, qs = c // 4, (c % 4) * LQL
        sp[b, qs:qs + LQL] = res.results[c]["out_s"]
        op[b, qs:qs + LQL] = res.results[c]["out_o"]
    return sp, op


# revision 19
# speedup vs baseline: 1.0052x; 1.0052x over previous
"""Trainium2 Bass kernel for nn_MultiHeadAttention_59863254172512.

Sharding: 8 cores = 2 batches x 4 query-row blocks of 512. Each core computes
the full attention for its (batch, 512 query rows): k/v projections for the
whole batch are recomputed per core (no cross-core communication needed).

Layout strategy per core:
  - activations are DMA-transposed (XBAR, bf16) into [D-part, L] tiles
  - q/k projected into transposed layout [HDK-part, L]; v_s/v_o into natural
    [LK-part, DV] (v_s augmented with a ones column per head for the softmax
    denominator)
  - scores computed transposed [LK-part, LQ]; exp on ScalarE with the
    per-key bias and 1/16 scale folded in; mask applied multiplicatively
  - attn@v contracts keys on partitions; denominator rides the ones column
  - normalization deferred (linearity of fc) to the o^T tiles via a
    DRAM-bounce partition-broadcast of 1/den
  - fc output lands natural [LQ-part, D]; residual add + LayerNorm on-chip
"""

from contextlib import ExitStack

import numpy as np
import ml_dtypes

import concourse.bass as bass
import concourse.mybir as mybir
import concourse.tile as tile
from concourse import bacc, bass_utils

B, LQ, LK, D = 2, 2048, 2048, 1024
H, DK, DV = 16, 64, 64
EPS = 1e-6
P = 128
N_CORES = 8
LQL = LQ * B // N_CORES          # 512 query rows per core
DC = D // P                      # 8 contraction chunks
KC = LK // P                     # 16 key chunks
HC = (H * DK) // P               # 8 hdk chunks (2 heads per chunk)
LCH = LQL // P                   # 4 local query row chunks
F32 = mybir.dt.float32
BF16 = mybir.dt.bfloat16

_CACHE = {}
DEBUG = False


def _build_nc():
    nc = bacc.Bacc("TRN2", target_bir_lowering=False, debug=False,
                   num_devices=N_CORES)

    # ---- I/O ----
    gs_bf = nc.dram_tensor("gs_bf", [LQL, D], BF16, kind="ExternalInput")
    xk_bf = nc.dram_tensor("xk_bf", [LK, D], BF16, kind="ExternalInput")
    xs_bf = nc.dram_tensor("xs_bf", [LK, D], BF16, kind="ExternalInput")
    xo_bf = nc.dram_tensor("xo_bf", [LK, D], BF16, kind="ExternalInput")
    mask_bf = nc.dram_tensor("mask_bf", [LQL, LK], BF16, kind="ExternalInput")
    hbias = nc.dram_tensor("hbias", [LK], F32, kind="ExternalInput")
    resid = nc.dram_tensor("resid", [LQL, D], F32, kind="ExternalInput")
    wq_bf = nc.dram_tensor("wq_bf", [D, H * DK], BF16, kind="ExternalInput")
    wk_bf = nc.dram_tensor("wk_bf", [D, H * DK], BF16, kind="ExternalInput")
    wv_bf = nc.dram_tensor("wv_bf", [D, H * DV], BF16, kind="ExternalInput")
    wfc_bf = nc.dram_tensor("wfc_bf", [H * DV, D], BF16, kind="ExternalInput")
    gamma = nc.dram_tensor("gamma", [D], F32, kind="ExternalInput")
    beta = nc.dram_tensor("beta", [D], F32, kind="ExternalInput")
    out_s = nc.dram_tensor("out_s", [LQL, D], F32, kind="ExternalOutput")
    out_o = nc.dram_tensor("out_o", [LQL, D], F32, kind="ExternalOutput")
    if DEBUG:
        dbg_den = nc.dram_tensor("dbg_den", [H, LQL], F32, kind="ExternalOutput")
        dbg_os = nc.dram_tensor("dbg_os", [P, HC, LQL], BF16, kind="ExternalOutput")
        dbg_q = nc.dram_tensor("dbg_q", [P, HC, LQL], BF16, kind="ExternalOutput")

    with tile.TileContext(nc) as tc, ExitStack() as ctx:
        if True:
            const_p = ctx.enter_context(tc.tile_pool(name="const", bufs=1))
            xt_p = ctx.enter_context(tc.tile_pool(name="xt", bufs=3))
            w_p = ctx.enter_context(tc.tile_pool(name="w", bufs=2))
            pers_p = ctx.enter_context(tc.tile_pool(name="persist", bufs=1))
            aug_p = ctx.enter_context(tc.tile_pool(name="aug", bufs=2))
            vh_p = ctx.enter_context(tc.tile_pool(name="vh", bufs=2))
            attn_p = ctx.enter_context(tc.tile_pool(name="attn", bufs=6))
            den_p = ctx.enter_context(tc.tile_pool(name="den", bufs=2))
            res_p = ctx.enter_context(tc.tile_pool(name="resp", bufs=1))
            ln_p = ctx.enter_context(tc.tile_pool(name="lnp", bufs=3))
            stats_p = ctx.enter_context(tc.tile_pool(name="stats", bufs=4))
            ps_mm = ctx.enter_context(tc.tile_pool(name="psmm", bufs=2, space="PSUM"))
            ps_sc = ctx.enter_context(tc.tile_pool(name="pssc", bufs=2, space="PSUM"))
            ps_acc = ctx.enter_context(tc.tile_pool(name="psacc", bufs=2, space="PSUM"))
            dram_p = ctx.enter_context(tc.tile_pool(name="dram", bufs=1, space="DRAM"))
            dram_den = ctx.enter_context(tc.tile_pool(name="dramd", bufs=2, space="DRAM"))
            # ---- persistent attention operands ----
            kT = pers_p.tile([P, HC, LK], BF16)      # 4 MiB
            qT = pers_p.tile([P, HC, LQL], BF16)     # 1 MiB
            maskT = pers_p.tile([P, KC, LQL], BF16)  # 2 MiB
            osT = pers_p.tile([P, HC, LQL], BF16)    # 1 MiB
            ooT = pers_p.tile([P, HC, LQL], BF16)    # 1 MiB

            # DRAM scratch for v (streamed back per head)
            vsD = dram_p.tile([KC, P, H * DV], BF16)
            voD = dram_p.tile([KC, P, H * DV], BF16)


            class XTPair:
                def __init__(self, a, b):
                    self._t = (a, b)

                def __getitem__(self, key):
                    p, dc, fs = key
                    return self._t[dc // 4][p, dc % 4, fs]

            def load_xt(src):
                t0 = xt_p.tile([P, 4, LK], BF16, tag="xt", name="xt_t")
                for dc in range(4):
                    nc.sync.dma_start_transpose(
                        t0[:, dc, :], src[:, dc * P:(dc + 1) * P])
                t1 = xt_p.tile([P, 4, LK], BF16, tag="xt", name="xt_t")
                for dc in range(4, DC):
                    nc.sync.dma_start_transpose(
                        t1[:, dc - 4, :], src[:, dc * P:(dc + 1) * P])
                return XTPair(t0, t1)

            def load_w(src, n):
                t = w_p.tile([P, DC, 1024], BF16, tag="w", name="w_t")
                nc.gpsimd.dma_start(t[:], src.rearrange("(o p) n -> p o n", p=P))
                return t

            # ---- phase B: projections ----
            wk_sb = load_w(wk_bf, "wk")
            xkT = load_xt(xk_bf)
            wq_sb = load_w(wq_bf, "wq")
            xqT = xt_p.tile([P, DC, LQL], BF16, tag="xt", name="xqT")
            for dc in range(DC):
                nc.sync.dma_start_transpose(
                    xqT[:, dc, :], gs_bf[:, dc * P:(dc + 1) * P])
            for kc in range(KC):
                nc.sync.dma_start_transpose(
                    maskT[:, kc, :], mask_bf[:, kc * P:(kc + 1) * P])

            # k projection -> kT
            for hc in range(HC):
                for lc in range(LK // 512):
                    ps_prj = ps_mm.tile([P, 512], F32, tag="mm", name="ps_prj")
                    for dc in range(DC):
                        nc.tensor.matmul(
                            ps_prj[:],
                            wk_sb[:, dc, hc * P:(hc + 1) * P],
                            xkT[slice(None), dc, slice(lc * 512, (lc + 1) * 512)],
                            start=(dc == 0), stop=(dc == DC - 1))
                    nc.any.tensor_copy(out=kT[:, hc, lc * 512:(lc + 1) * 512],
                                       in_=ps_prj[:])

            # q projection -> qT (runs while XsT transposes stream in)
            for hc in range(HC):
                ps_prj = ps_mm.tile([P, 512], F32, tag="mm", name="ps_prj")
                for dc in range(DC):
                    nc.tensor.matmul(ps_prj[:, :LQL],
                                     wq_sb[:, dc, hc * P:(hc + 1) * P],
                                     xqT[:, dc, :],
                                     start=(dc == 0), stop=(dc == DC - 1))
                nc.any.tensor_copy(out=qT[:, hc, :], in_=ps_prj[:, :LQL])

            # v_s projection -> DRAM
            xsT = load_xt(xs_bf)
            wv_sb = load_w(wv_bf, "wv")
            for kc in range(KC):
                vs_t = aug_p.tile([P, H * DV], BF16, tag="vo_t", name="vs_t")
                for dvc in range(2):
                    ps_prj = ps_mm.tile([P, 512], F32, tag="mm", name="ps_prj")
                    for dc in range(DC):
                        nc.tensor.matmul(
                            ps_prj[:],
                            xsT[slice(None), dc, slice(kc * P, (kc + 1) * P)],
                            wv_sb[:, dc, dvc * 512:(dvc + 1) * 512],
                            start=(dc == 0), stop=(dc == DC - 1))
                    nc.vector.tensor_copy(out=vs_t[:, dvc * 512:(dvc + 1) * 512],
                                       in_=ps_prj[:])
                nc.scalar.dma_start(vsD[kc], vs_t[:])

            # v_o projection -> DRAM
            xoT = load_xt(xo_bf)
            for kc in range(KC):
                vo_t = aug_p.tile([P, H * DV], BF16, tag="vo_t", name="vo_t")
                for dvc in range(2):
                    ps_prj = ps_mm.tile([P, 512], F32, tag="mm", name="ps_prj")
                    for dc in range(DC):
                        nc.tensor.matmul(
                            ps_prj[:],
                            xoT[slice(None), dc, slice(kc * P, (kc + 1) * P)],
                            wv_sb[:, dc, dvc * 512:(dvc + 1) * 512],
                            start=(dc == 0), stop=(dc == DC - 1))
                    nc.vector.tensor_copy(out=vo_t[:, dvc * 512:(dvc + 1) * 512],
                                       in_=ps_prj[:])
                nc.scalar.dma_start(voD[kc], vo_t[:])

            wfc_sb = load_w(wfc_bf, "wfc")

            # ---- constants (needed from attention phase on) ----
            hb_sb = const_p.tile([P, KC], F32)
            nc.gpsimd.dma_start(hb_sb[:], hbias.rearrange("(o p) -> p o", p=P))
            gamma_bc = const_p.tile([P, D], F32)
            nc.gpsimd.dma_start(gamma_bc[:], gamma[:].partition_broadcast(P))
            beta_bc = const_p.tile([P, D], F32)
            nc.gpsimd.dma_start(beta_bc[:], beta[:].partition_broadcast(P))
            eps_sb = const_p.tile([P, 1], F32)
            nc.vector.memset(eps_sb[:], EPS)
            ones_sb = const_p.tile([P, 1], BF16)
            nc.vector.memset(ones_sb[:], 1.0)

            # ---- phase C: attention per head ----
            for h in range(H):
                hp, hr = h // 2, (h % 2) * 64
                sv_h = vh_p.tile([P, KC, 2 * DV], BF16)
                nc.sync.dma_start(
                    sv_h[:, :, 0:DV], vsD[:, :, h * DV:(h + 1) * DV]
                    .rearrange("k p c -> p k c"))
                nc.sync.dma_start(
                    sv_h[:, :, DV:2 * DV], voD[:, :, h * DV:(h + 1) * DV]
                    .rearrange("k p c -> p k c"))

                ps_sv = ps_acc.tile([P, 512], F32, tag="ps_sv", name="ps_sv")
                den_ps = ps_acc.tile([P, 4], F32, tag="den", name="den_ps")
                for kc in range(KC):
                    ps_scr = ps_sc.tile([P, 512], F32, tag="sc", name="ps_scr")
                    nc.tensor.matmul(
                        ps_scr[:, :LQL],
                        kT[hr:hr + 64, hp, kc * P:(kc + 1) * P],
                        qT[hr:hr + 64, hp, :],
                        start=True, stop=True)
                    attn_t = attn_p.tile([P, LQL], BF16)
                    nc.scalar.activation(
                        out=attn_t[:], in_=ps_scr[:, :LQL],
                        func=mybir.ActivationFunctionType.Exp,
                        bias=hb_sb[:, kc:kc + 1], scale=0.0625)
                    nc.vector.tensor_mul(
                        out=attn_t[:], in0=attn_t[:], in1=maskT[:, kc, :])
                    nc.tensor.matmul(ps_sv[:, :LQL], sv_h[:, kc, :], attn_t[:],
                                     start=(kc == 0), stop=(kc == KC - 1))
                    for qc in range(4):
                        nc.tensor.matmul(
                            den_ps[:, qc:qc + 1],
                            attn_t[:, qc * P:(qc + 1) * P], ones_sb[:],
                            start=(kc == 0 and qc == 0),
                            stop=(kc == KC - 1 and qc == 3))

                # denominator -> reciprocal -> broadcast over 64 partitions
                rc4 = den_p.tile([P, 4], F32)
                nc.vector.reciprocal(out=rc4[:], in_=den_ps[:])
                denD = dram_den.tile([LQL], F32)
                nc.sync.dma_start(
                    denD.rearrange("(q p) -> p q", p=P), rc4[:])
                recip_bc = den_p.tile([64, LQL], F32)
                nc.sync.dma_start(recip_bc[:], denD[:].partition_broadcast(64))

                nc.vector.tensor_mul(out=osT[hr:hr + 64, hp, :],
                                     in0=ps_sv[0:64, :LQL], in1=recip_bc[:])
                nc.vector.tensor_mul(out=ooT[hr:hr + 64, hp, :],
                                     in0=ps_sv[64:128, :LQL], in1=recip_bc[:])
                if DEBUG:
                    nc.sync.dma_start(dbg_den[h:h + 1, :],
                                      denD[:].partition_broadcast(1))

            if DEBUG:
                nc.sync.dma_start(dbg_q[:], qT[:])
                nc.sync.dma_start(dbg_os[:], osT[:])

            # ---- phase D: fc + residual + layernorm ----
            for lc in range(LCH):
                res_t = res_p.tile([P, D], F32)
                nc.gpsimd.dma_start(res_t[:], resid[lc * P:(lc + 1) * P, :])
                for variant, (oT, out_d) in enumerate(
                        [(osT, out_s), (ooT, out_o)]):
                    ln_t = ln_p.tile([P, D], F32)
                    for dc2 in range(2):
                        ps_fc = ps_mm.tile([P, 512], F32, tag="mm", name="ps_fc")
                        for hp2 in range(HC):
                            nc.tensor.matmul(
                                ps_fc[:],
                                oT[:, hp2, lc * P:(lc + 1) * P],
                                wfc_sb[:, hp2, dc2 * 512:(dc2 + 1) * 512],
                                start=(hp2 == 0), stop=(hp2 == HC - 1))
                        nc.vector.tensor_add(
                            out=ln_t[:, dc2 * 512:(dc2 + 1) * 512],
                            in0=ps_fc[:],
                            in1=res_t[:, dc2 * 512:(dc2 + 1) * 512])
                    # layernorm over D
                    stats = stats_p.tile([P, 2, nc.vector.BN_STATS_DIM], F32)
                    for sg in range(2):
                        nc.vector.bn_stats(
                            out=stats[:, sg, :],
                            in_=ln_t[:, sg * 512:(sg + 1) * 512])
                    mv = stats_p.tile([P, nc.vector.BN_AGGR_DIM], F32)
                    nc.vector.bn_aggr(out=mv[:], in_=stats[:])
                    rstd = stats_p.tile([P, 1], F32)
                    nc.scalar.activation(
                        out=rstd[:], in_=mv[:, 1:2],
                        func=mybir.ActivationFunctionType.Sqrt,
                        bias=eps_sb[:], scale=1.0)
                    nc.vector.reciprocal(out=rstd[:], in_=rstd[:])
                    nc.vector.tensor_scalar(
                        out=ln_t[:], in0=ln_t[:],
                        scalar1=mv[:, 0:1], scalar2=rstd[:],
                        op0=mybir.AluOpType.subtract,
                        op1=mybir.AluOpType.mult)
                    nc.vector.tensor_mul(out=ln_t[:], in0=ln_t[:],
                                         in1=gamma_bc[:])
                    nc.vector.tensor_add(out=ln_t[:], in0=ln_t[:],
                                         in1=beta_bc[:])
                    nc.sync.dma_start(out_d[lc * P:(lc + 1) * P, :], ln_t[:])

    nc.compile()
    return nc


def _in_maps(PE_states, global_state, PE_statements, PE_operators,
             PE_solution_scores, mask, w_q, w_k, w_v, w_fc, ln_gamma, ln_beta):
    bf = lambda a: np.ascontiguousarray(a).astype(ml_dtypes.bfloat16)
    f32 = lambda a: np.ascontiguousarray(a, dtype=np.float32)
    xk = [bf(PE_states[b]) for b in range(B)]
    xs = [bf(PE_statements[b]) for b in range(B)]
    xo = [bf(PE_operators[b]) for b in range(B)]
    hb = [f32(PE_solution_scores[b] * 0.5) for b in range(B)]
    wq, wk, wv, wfc = bf(w_q), bf(w_k), bf(w_v), bf(w_fc)
    g, bt = f32(ln_gamma), f32(ln_beta)
    maps = []
    for c in range(N_CORES):
        b, qs = c // 4, (c % 4) * LQL
        maps.append({
            "gs_bf": bf(global_state[b, qs:qs + LQL]),
            "xk_bf": xk[b], "xs_bf": xs[b], "xo_bf": xo[b],
            "mask_bf": bf(mask[b, qs:qs + LQL]),
            "hbias": hb[b],
            "resid": f32(global_state[b, qs:qs + LQL]),
            "wq_bf": wq, "wk_bf": wk, "wv_bf": wv, "wfc_bf": wfc,
            "gamma": g, "beta": bt,
        })
    return maps


def kernel(**inputs):
    if "nc" not in _CACHE:
        _CACHE["nc"] = _build_nc()
    nc = _CACHE["nc"]
    maps = _in_maps(**inputs)
    res = bass_utils.run_bass_kernel_spmd(nc, maps, core_ids=list(range(N_CORES)))
    sp = np.empty((B, LQ, D), np.float32)
    op = np.empty((B, LQ, D), np.float32)
    for c in range(N_CORES):
        b, qs = c // 4, (c % 4) * LQL
        sp[b, qs:qs + LQL] = res.results[c]["out_s"]
        op[b, qs:qs + LQL] = res.results[c]["out_o"]
    return sp, op


# revision 20
# speedup vs baseline: 1.0088x; 1.0035x over previous
"""Trainium2 Bass kernel for nn_MultiHeadAttention_59863254172512.

Sharding: 8 cores = 2 batches x 4 query-row blocks of 512. Each core computes
the full attention for its (batch, 512 query rows): k/v projections for the
whole batch are recomputed per core (no cross-core communication needed).

Layout strategy per core:
  - activations are DMA-transposed (XBAR, bf16) into [D-part, L] tiles
  - q/k projected into transposed layout [HDK-part, L]; v_s/v_o into natural
    [LK-part, DV] (v_s augmented with a ones column per head for the softmax
    denominator)
  - scores computed transposed [LK-part, LQ]; exp on ScalarE with the
    per-key bias and 1/16 scale folded in; mask applied multiplicatively
  - attn@v contracts keys on partitions; denominator rides the ones column
  - normalization deferred (linearity of fc) to the o^T tiles via a
    DRAM-bounce partition-broadcast of 1/den
  - fc output lands natural [LQ-part, D]; residual add + LayerNorm on-chip
"""

from contextlib import ExitStack

import numpy as np
import ml_dtypes

import concourse.bass as bass
import concourse.mybir as mybir
import concourse.tile as tile
from concourse import bacc, bass_utils

B, LQ, LK, D = 2, 2048, 2048, 1024
H, DK, DV = 16, 64, 64
EPS = 1e-6
P = 128
N_CORES = 8
LQL = LQ * B // N_CORES          # 512 query rows per core
DC = D // P                      # 8 contraction chunks
KC = LK // P                     # 16 key chunks
HC = (H * DK) // P               # 8 hdk chunks (2 heads per chunk)
LCH = LQL // P                   # 4 local query row chunks
F32 = mybir.dt.float32
BF16 = mybir.dt.bfloat16

_CACHE = {}
DEBUG = False


def _build_nc():
    nc = bacc.Bacc("TRN2", target_bir_lowering=False, debug=False,
                   num_devices=N_CORES)

    # ---- I/O ----
    gs_bf = nc.dram_tensor("gs_bf", [LQL, D], BF16, kind="ExternalInput")
    xk_bf = nc.dram_tensor("xk_bf", [LK, D], BF16, kind="ExternalInput")
    xs_bf = nc.dram_tensor("xs_bf", [LK, D], BF16, kind="ExternalInput")
    xo_bf = nc.dram_tensor("xo_bf", [LK, D], BF16, kind="ExternalInput")
    mask_bf = nc.dram_tensor("mask_bf", [LQL, LK], BF16, kind="ExternalInput")
    hbias = nc.dram_tensor("hbias", [LK], F32, kind="ExternalInput")
    resid = nc.dram_tensor("resid", [LQL, D], F32, kind="ExternalInput")
    wq_bf = nc.dram_tensor("wq_bf", [D, H * DK], BF16, kind="ExternalInput")
    wk_bf = nc.dram_tensor("wk_bf", [D, H * DK], BF16, kind="ExternalInput")
    wv_bf = nc.dram_tensor("wv_bf", [D, H * DV], BF16, kind="ExternalInput")
    wfc_bf = nc.dram_tensor("wfc_bf", [H * DV, D], BF16, kind="ExternalInput")
    gamma = nc.dram_tensor("gamma", [D], F32, kind="ExternalInput")
    beta = nc.dram_tensor("beta", [D], F32, kind="ExternalInput")
    out_s = nc.dram_tensor("out_s", [LQL, D], F32, kind="ExternalOutput")
    out_o = nc.dram_tensor("out_o", [LQL, D], F32, kind="ExternalOutput")
    if DEBUG:
        dbg_den = nc.dram_tensor("dbg_den", [H, LQL], F32, kind="ExternalOutput")
        dbg_os = nc.dram_tensor("dbg_os", [P, HC, LQL], BF16, kind="ExternalOutput")
        dbg_q = nc.dram_tensor("dbg_q", [P, HC, LQL], BF16, kind="ExternalOutput")

    with tile.TileContext(nc) as tc, ExitStack() as ctx:
        if True:
            const_p = ctx.enter_context(tc.tile_pool(name="const", bufs=1))
            xt_p = ctx.enter_context(tc.tile_pool(name="xt", bufs=3))
            w_p = ctx.enter_context(tc.tile_pool(name="w", bufs=2))
            pers_p = ctx.enter_context(tc.tile_pool(name="persist", bufs=1))
            aug_p = ctx.enter_context(tc.tile_pool(name="aug", bufs=2))
            vh_p = ctx.enter_context(tc.tile_pool(name="vh", bufs=2))
            attn_p = ctx.enter_context(tc.tile_pool(name="attn", bufs=6))
            den_p = ctx.enter_context(tc.tile_pool(name="den", bufs=2))
            res_p = ctx.enter_context(tc.tile_pool(name="resp", bufs=1))
            ln_p = ctx.enter_context(tc.tile_pool(name="lnp", bufs=3))
            stats_p = ctx.enter_context(tc.tile_pool(name="stats", bufs=4))
            ps_mm = ctx.enter_context(tc.tile_pool(name="psmm", bufs=2, space="PSUM"))
            ps_sc = ctx.enter_context(tc.tile_pool(name="pssc", bufs=3, space="PSUM"))
            ps_acc = ctx.enter_context(tc.tile_pool(name="psacc", bufs=2, space="PSUM"))
            ps_den = ctx.enter_context(tc.tile_pool(name="psden", bufs=1, space="PSUM"))
            dram_p = ctx.enter_context(tc.tile_pool(name="dram", bufs=1, space="DRAM"))
            dram_den = ctx.enter_context(tc.tile_pool(name="dramd", bufs=2, space="DRAM"))
            # ---- persistent attention operands ----
            kT = pers_p.tile([P, HC, LK], BF16)      # 4 MiB
            qT = pers_p.tile([P, HC, LQL], BF16)     # 1 MiB
            maskT = pers_p.tile([P, KC, LQL], BF16)  # 2 MiB
            osT = pers_p.tile([P, HC, LQL], BF16)    # 1 MiB
            ooT = pers_p.tile([P, HC, LQL], BF16)    # 1 MiB

            # DRAM scratch for v (streamed back per head)
            vsD = dram_p.tile([KC, P, H * DV], BF16)
            voD = dram_p.tile([KC, P, H * DV], BF16)


            class XTPair:
                def __init__(self, a, b):
                    self._t = (a, b)

                def __getitem__(self, key):
                    p, dc, fs = key
                    return self._t[dc // 4][p, dc % 4, fs]

            def load_xt(src):
                t0 = xt_p.tile([P, 4, LK], BF16, tag="xt", name="xt_t")
                for dc in range(4):
                    nc.sync.dma_start_transpose(
                        t0[:, dc, :], src[:, dc * P:(dc + 1) * P])
                t1 = xt_p.tile([P, 4, LK], BF16, tag="xt", name="xt_t")
                for dc in range(4, DC):
                    nc.sync.dma_start_transpose(
                        t1[:, dc - 4, :], src[:, dc * P:(dc + 1) * P])
                return XTPair(t0, t1)

            def load_w(src, n):
                t = w_p.tile([P, DC, 1024], BF16, tag="w", name="w_t")
                nc.gpsimd.dma_start(t[:], src.rearrange("(o p) n -> p o n", p=P))
                return t

            # ---- phase B: projections ----
            wk_sb = load_w(wk_bf, "wk")
            xkT = load_xt(xk_bf)
            wq_sb = load_w(wq_bf, "wq")
            xqT = xt_p.tile([P, DC, LQL], BF16, tag="xt", name="xqT")
            for dc in range(DC):
                nc.sync.dma_start_transpose(
                    xqT[:, dc, :], gs_bf[:, dc * P:(dc + 1) * P])
            for kc in range(KC):
                nc.sync.dma_start_transpose(
                    maskT[:, kc, :], mask_bf[:, kc * P:(kc + 1) * P])

            # k projection -> kT
            for hc in range(HC):
                for lc in range(LK // 512):
                    ps_prj = ps_mm.tile([P, 512], F32, tag="mm", name="ps_prj")
                    for dc in range(DC):
                        nc.tensor.matmul(
                            ps_prj[:],
                            wk_sb[:, dc, hc * P:(hc + 1) * P],
                            xkT[slice(None), dc, slice(lc * 512, (lc + 1) * 512)],
                            start=(dc == 0), stop=(dc == DC - 1))
                    nc.any.tensor_copy(out=kT[:, hc, lc * 512:(lc + 1) * 512],
                                       in_=ps_prj[:])

            # q projection -> qT (runs while XsT transposes stream in)
            for hc in range(HC):
                ps_prj = ps_mm.tile([P, 512], F32, tag="mm", name="ps_prj")
                for dc in range(DC):
                    nc.tensor.matmul(ps_prj[:, :LQL],
                                     wq_sb[:, dc, hc * P:(hc + 1) * P],
                                     xqT[:, dc, :],
                                     start=(dc == 0), stop=(dc == DC - 1))
                nc.any.tensor_copy(out=qT[:, hc, :], in_=ps_prj[:, :LQL])

            # v_s projection -> DRAM
            xsT = load_xt(xs_bf)
            wv_sb = load_w(wv_bf, "wv")
            for kc in range(KC):
                vs_t = aug_p.tile([P, H * DV], BF16, tag="vo_t", name="vs_t")
                for dvc in range(2):
                    ps_prj = ps_mm.tile([P, 512], F32, tag="mm", name="ps_prj")
                    for dc in range(DC):
                        nc.tensor.matmul(
                            ps_prj[:],
                            xsT[slice(None), dc, slice(kc * P, (kc + 1) * P)],
                            wv_sb[:, dc, dvc * 512:(dvc + 1) * 512],
                            start=(dc == 0), stop=(dc == DC - 1))
                    nc.vector.tensor_copy(out=vs_t[:, dvc * 512:(dvc + 1) * 512],
                                       in_=ps_prj[:])
                nc.scalar.dma_start(vsD[kc], vs_t[:])

            # v_o projection -> DRAM
            xoT = load_xt(xo_bf)
            for kc in range(KC):
                vo_t = aug_p.tile([P, H * DV], BF16, tag="vo_t", name="vo_t")
                for dvc in range(2):
                    ps_prj = ps_mm.tile([P, 512], F32, tag="mm", name="ps_prj")
                    for dc in range(DC):
                        nc.tensor.matmul(
                            ps_prj[:],
                            xoT[slice(None), dc, slice(kc * P, (kc + 1) * P)],
                            wv_sb[:, dc, dvc * 512:(dvc + 1) * 512],
                            start=(dc == 0), stop=(dc == DC - 1))
                    nc.vector.tensor_copy(out=vo_t[:, dvc * 512:(dvc + 1) * 512],
                                       in_=ps_prj[:])
                nc.scalar.dma_start(voD[kc], vo_t[:])

            wfc_sb = load_w(wfc_bf, "wfc")

            # ---- constants (needed from attention phase on) ----
            hb_sb = const_p.tile([P, KC], F32)
            nc.gpsimd.dma_start(hb_sb[:], hbias.rearrange("(o p) -> p o", p=P))
            gamma_bc = const_p.tile([P, D], F32)
            nc.gpsimd.dma_start(gamma_bc[:], gamma[:].partition_broadcast(P))
            beta_bc = const_p.tile([P, D], F32)
            nc.gpsimd.dma_start(beta_bc[:], beta[:].partition_broadcast(P))
            eps_sb = const_p.tile([P, 1], F32)
            nc.vector.memset(eps_sb[:], EPS)
            ones_sb = const_p.tile([P, 1], BF16)
            nc.vector.memset(ones_sb[:], 1.0)

            # ---- phase C: attention per head ----
            for h in range(H):
                hp, hr = h // 2, (h % 2) * 64
                sv_h = vh_p.tile([P, KC, 2 * DV], BF16)
                nc.sync.dma_start(
                    sv_h[:, :, 0:DV], vsD[:, :, h * DV:(h + 1) * DV]
                    .rearrange("k p c -> p k c"))
                nc.sync.dma_start(
                    sv_h[:, :, DV:2 * DV], voD[:, :, h * DV:(h + 1) * DV]
                    .rearrange("k p c -> p k c"))

                ps_sv = ps_acc.tile([P, 512], F32, tag="ps_sv", name="ps_sv")
                den_ps = ps_den.tile([P, 4], F32, tag="den", name="den_ps")
                for kc in range(KC):
                    ps_scr = ps_sc.tile([P, 512], F32, tag="sc", name="ps_scr")
                    nc.tensor.matmul(
                        ps_scr[:, :LQL],
                        kT[hr:hr + 64, hp, kc * P:(kc + 1) * P],
                        qT[hr:hr + 64, hp, :],
                        start=True, stop=True)
                    attn_t = attn_p.tile([P, LQL], BF16)
                    nc.scalar.activation(
                        out=attn_t[:], in_=ps_scr[:, :LQL],
                        func=mybir.ActivationFunctionType.Exp,
                        bias=hb_sb[:, kc:kc + 1], scale=0.0625)
                    nc.vector.tensor_mul(
                        out=attn_t[:], in0=attn_t[:], in1=maskT[:, kc, :])
                    nc.tensor.matmul(ps_sv[:, :LQL], sv_h[:, kc, :], attn_t[:],
                                     start=(kc == 0), stop=(kc == KC - 1))
                    for qc in range(4):
                        nc.tensor.matmul(
                            den_ps[:, qc:qc + 1],
                            attn_t[:, qc * P:(qc + 1) * P], ones_sb[:],
                            start=(kc == 0 and qc == 0),
                            stop=(kc == KC - 1 and qc == 3))

                # denominator -> reciprocal -> broadcast over 64 partitions
                rc4 = den_p.tile([P, 4], F32)
                nc.vector.reciprocal(out=rc4[:], in_=den_ps[:])
                denD = dram_den.tile([LQL], F32)
                nc.sync.dma_start(
                    denD.rearrange("(q p) -> p q", p=P), rc4[:])
                recip_bc = den_p.tile([64, LQL], F32)
                nc.sync.dma_start(recip_bc[:], denD[:].partition_broadcast(64))

                nc.vector.tensor_mul(out=osT[hr:hr + 64, hp, :],
                                     in0=ps_sv[0:64, :LQL], in1=recip_bc[:])
                nc.vector.tensor_mul(out=ooT[hr:hr + 64, hp, :],
                                     in0=ps_sv[64:128, :LQL], in1=recip_bc[:])
                if DEBUG:
                    nc.sync.dma_start(dbg_den[h:h + 1, :],
                                      denD[:].partition_broadcast(1))

            if DEBUG:
                nc.sync.dma_start(dbg_q[:], qT[:])
                nc.sync.dma_start(dbg_os[:], osT[:])

            # ---- phase D: fc + residual + layernorm ----
            for lc in range(LCH):
                res_t = res_p.tile([P, D], F32)
                nc.gpsimd.dma_start(res_t[:], resid[lc * P:(lc + 1) * P, :])
                for variant, (oT, out_d) in enumerate(
                        [(osT, out_s), (ooT, out_o)]):
                    ln_t = ln_p.tile([P, D], F32)
                    for dc2 in range(2):
                        ps_fc = ps_mm.tile([P, 512], F32, tag="mm", name="ps_fc")
                        for hp2 in range(HC):
                            nc.tensor.matmul(
                                ps_fc[:],
                                oT[:, hp2, lc * P:(lc + 1) * P],
                                wfc_sb[:, hp2, dc2 * 512:(dc2 + 1) * 512],
                                start=(hp2 == 0), stop=(hp2 == HC - 1))
                        nc.vector.tensor_add(
                            out=ln_t[:, dc2 * 512:(dc2 + 1) * 512],
                            in0=ps_fc[:],
                            in1=res_t[:, dc2 * 512:(dc2 + 1) * 512])
                    # layernorm over D
                    stats = stats_p.tile([P, 2, nc.vector.BN_STATS_DIM], F32)
                    for sg in range(2):
                        nc.vector.bn_stats(
                            out=stats[:, sg, :],
                            in_=ln_t[:, sg * 512:(sg + 1) * 512])
                    mv = stats_p.tile([P, nc.vector.BN_AGGR_DIM], F32)
                    nc.vector.bn_aggr(out=mv[:], in_=stats[:])
                    rstd = stats_p.tile([P, 1], F32)
                    nc.scalar.activation(
                        out=rstd[:], in_=mv[:, 1:2],
                        func=mybir.ActivationFunctionType.Sqrt,
                        bias=eps_sb[:], scale=1.0)
                    nc.vector.reciprocal(out=rstd[:], in_=rstd[:])
                    nc.vector.tensor_scalar(
                        out=ln_t[:], in0=ln_t[:],
                        scalar1=mv[:, 0:1], scalar2=rstd[:],
                        op0=mybir.AluOpType.subtract,
                        op1=mybir.AluOpType.mult)
                    nc.vector.tensor_mul(out=ln_t[:], in0=ln_t[:],
                                         in1=gamma_bc[:])
                    nc.vector.tensor_add(out=ln_t[:], in0=ln_t[:],
                                         in1=beta_bc[:])
                    nc.sync.dma_start(out_d[lc * P:(lc + 1) * P, :], ln_t[:])

    nc.compile()
    return nc


def _in_maps(PE_states, global_state, PE_statements, PE_operators,
             PE_solution_scores, mask, w_q, w_k, w_v, w_fc, ln_gamma, ln_beta):
    bf = lambda a: np.ascontiguousarray(a).astype(ml_dtypes.bfloat16)
    f32 = lambda a: np.ascontiguousarray(a, dtype=np.float32)
    xk = [bf(PE_states[b]) for b in range(B)]
    xs = [bf(PE_statements[b]) for b in range(B)]
    xo = [bf(PE_operators[b]) for b in range(B)]
    hb = [f32(PE_solution_scores[b] * 0.5) for b in range(B)]
    wq, wk, wv, wfc = bf(w_q), bf(w_k), bf(w_v), bf(w_fc)
    g, bt = f32(ln_gamma), f32(ln_beta)
    maps = []
    for c in range(N_CORES):
        b, qs = c // 4, (c % 4) * LQL
        maps.append({
            "gs_bf": bf(global_state[b, qs:qs + LQL]),
            "xk_bf": xk[b], "xs_bf": xs[b], "xo_bf": xo[b],
            "mask_bf": bf(mask[b, qs:qs + LQL]),
            "hbias": hb[b],
            "resid": f32(global_state[b, qs:qs + LQL]),
            "wq_bf": wq, "wk_bf": wk, "wv_bf": wv, "wfc_bf": wfc,
            "gamma": g, "beta": bt,
        })
    return maps


def kernel(**inputs):
    if "nc" not in _CACHE:
        _CACHE["nc"] = _build_nc()
    nc = _CACHE["nc"]
    maps = _in_maps(**inputs)
    res = bass_utils.run_bass_kernel_spmd(nc, maps, core_ids=list(range(N_CORES)))
    sp = np.empty((B, LQ, D), np.float32)
    op = np.empty((B, LQ, D), np.float32)
    for c in range(N_CORES):
        b, qs = c // 4, (c % 4) * LQL
        sp[b, qs:qs + LQL] = res.results[c]["out_s"]
        op[b, qs:qs + LQL] = res.results[c]["out_o"]
    return sp, op
